# revision 3
# baseline (speedup 1.0000x reference)
"""LConv (7x7 position-linear conv) Trainium2 Bass kernel.

Full inputs in, full output out. Sharding: data-parallel over batch,
16 images -> 8 NeuronCores (2 images/core). abc/bias replicated.

Math: the 7x7 kernel weight is linear in position:
  w[u,v,c,o] = u*A[c,o] + v*B[c,o] + C[c,o]   (u,v in -3..3)
With R = box7 along W of x and q = box7 along H of x:
  out[o,i,j] = sum_u (u*A + C)^T R[., i+u, j]          (7 R-taps)
             + sum_{v in +-1,+-2} v*B^T q[., i, j+v]   (4 direct taps)
             + 3*B^T (q[.,i,j+3] - q[.,i,j-3])         (1 d3-map tap)
             + bias[o]
12 matmul taps per 4-row output tile: the v=+-3 pair collapses into
one tap over a difference map d3 = q[j+3]-q[j-3], built in two
row-halves with a trivial custom-DVE op (body = Src0 - Src1): stock
vector.tensor_tensor writes get NO dependency edge to tensor-engine
readers (trace-proven race), while custom-DVE writes are tracked.

R: sliding-box via the BOXDIFF custom-DVE op (cumsum(in0)-cumsum(in1))
   on the row-major stream; stored for rows 7..68 only (62 rows).
q: same op on a column-major (transposed) view into column pages
   (full 69-row pages - each page's in0-in1 must telescope to zero),
   then a scalar-engine relayout to row-major qg.

The image pair is processed as 4 half-image units (56 out rows each,
+/-3-row halo) with slot parity so DMA, scans, relayout, d3 and
matmuls pipeline across units. Odd input-DMA slices ride the gpsimd
queue: a multi-us direct DMA on the scalar queue would delay the
activations that free PSUM banks for matmul groups 3-4. Weights/maps
in bf16 (x stays f32: a bf16 HBM->SBUF strided DMA corrupts data on
this stack).
"""

import numpy as np

import concourse.bacc as bacc
import concourse.mybir as mybir
from concourse import tile
from concourse.bass_utils import run_bass_kernel_spmd

F32 = mybir.dt.float32
BF16 = mybir.dt.bfloat16
AF = mybir.ActivationFunctionType
ALU = mybir.AluOpType

B_TOT = 16
N_CORES = 8
B_PER = B_TOT // N_CORES
CIN = 128
COUT = 128
H = W = 112
PW2 = 122                  # 7 lead + 112 + 3 trail cols
UROWS = 56                 # output rows per unit (half image)
XROWS = 76                 # 7 lead + 62 (56+halo) + 7 trail rows
XBF = XROWS * PW2          # 9272
RROWS = 62                 # stored R rows (xb rows 7..68)
RBF = RROWS * PW2
DROWS = 59                 # valid x rows DMA'd per unit
QK = XROWS - 7             # 69 scanned values per column page
OUT_ROWS = 4
OTF = OUT_ROWS * W         # 448
NTAPS = 12                 # 7 R-taps + 4 direct v=+-1,+-2 taps + d3 tap
# R-scan chunks in xb rows (row-aligned; chunk 0 covers group-1 needs)
RCHUNKS = ((7, 34), (41, 28))


_CACHE = {}


def _register_boxdiff():
    from concourse.dve_spec import Spec, scan, AluOp, Src0, Src1, lower
    import concourse.dve_ops as dve_ops
    from concourse.dve_uop import DveOpSpec

    if any(op.name == "BOXDIFF7" for op in dve_ops.OPS):
        return next(op for op in dve_ops.OPS if op.name == "BOXDIFF7")
    spec = Spec(
        body=scan(AluOp.ADD, Src0) - scan(AluOp.ADD, Src1),
        reference=lambda in0, in1: (
            np.cumsum(in0, axis=-1) - np.cumsum(in1, axis=-1)
        ),
    )
    row = dve_ops._CUSTOM_DVE_ROW_BASE + len(dve_ops.OPS)
    shas = {}
    for ver in ("v3", "v4"):
        s = DveOpSpec(
            name="BOXDIFF7", opcode=row, uops=lower(spec, ver=ver), rd1_en=True
        )
        shas[ver] = s.sha(ver)
    op = dve_ops.DveOp("BOXDIFF7", spec, subdim=False, uops_sha=shas)
    dve_ops.OPS.append(op)
    dve_ops._SUB_OPCODE_FOR_NAME[op.name] = row
    dve_ops.CUSTOM_DVE_SPECS[op.name] = op.spec
    return op


def _register_subdiff():
    from concourse.dve_spec import Spec, Src0, Src1, lower
    import concourse.dve_ops as dve_ops
    from concourse.dve_uop import DveOpSpec

    if any(op.name == "SUBDIFF" for op in dve_ops.OPS):
        return next(op for op in dve_ops.OPS if op.name == "SUBDIFF")
    spec = Spec(
        body=Src0 - Src1,
        reference=lambda in0, in1: in0 - in1,
    )
    row = dve_ops._CUSTOM_DVE_ROW_BASE + len(dve_ops.OPS)
    shas = {}
    for ver in ("v3", "v4"):
        s = DveOpSpec(
            name="SUBDIFF", opcode=row, uops=lower(spec, ver=ver), rd1_en=True
        )
        shas[ver] = s.sha(ver)
    op = dve_ops.DveOp("SUBDIFF", spec, subdim=False, uops_sha=shas)
    dve_ops.OPS.append(op)
    dve_ops._SUB_OPCODE_FOR_NAME[op.name] = row
    dve_ops.CUSTOM_DVE_SPECS[op.name] = op.spec
    return op


def _build():
    nc = bacc.Bacc("TRN2", target_bir_lowering=False, debug=False)
    opa = _register_boxdiff()
    opd = _register_subdiff()

    t_x = nc.dram_tensor("xs", [B_PER, CIN, H, W], F32, kind="ExternalInput")
    t_w = nc.dram_tensor("wts", [NTAPS, CIN, COUT], F32, kind="ExternalInput")
    t_bias = nc.dram_tensor("bias", [COUT, 1], F32, kind="ExternalInput")
    t_out = nc.dram_tensor("out", [B_PER, COUT, H, W], BF16, kind="ExternalOutput")

    with tile.TileContext(nc) as tc:
        with (
            tc.tile_pool(name="const", bufs=1) as cpool,
            tc.tile_pool(name="bufs", bufs=1) as bpool,
            tc.tile_pool(name="outs", bufs=4) as opool,
            tc.tile_pool(name="ps", bufs=1, space="PSUM") as ppool,
        ):
            # ---- constants ----
            wf = cpool.tile([CIN, NTAPS * COUT], F32, tag="wf", name="wf")
            nc.scalar.dma_start(
                wf[:].rearrange("c (t o) -> c t o", t=NTAPS),
                t_w[:].transpose([1, 0, 2]),
            )
            wt = cpool.tile([CIN, NTAPS * COUT], BF16, tag="wt", name="wt")
            nc.vector.tensor_copy(wt[:], wf[:])
            bias_sb = cpool.tile([COUT, 1], F32, tag="bias", name="bias")
            nc.scalar.dma_start(bias_sb[:], t_bias[:])

            # ---- per-slot buffers (slot = unit parity = top/bottom) ----
            xbufs, rbufs, qgbufs, dvbufs = [], [], [], []
            qp = bpool.tile([CIN, 115 * QK], BF16, tag="qp", name="qp")
            for s in range(2):
                xb = bpool.tile([CIN, XBF], F32, tag=f"xb{s}", name=f"xb{s}")
                xv = xb[:].rearrange("c (r q) -> c r q", q=PW2)
                nc.gpsimd.memset(xb[:, : 7 * PW2], 0.0)            # lead rows
                nc.gpsimd.memset(xb[:, (XROWS - 7) * PW2 :], 0.0)  # trail rows
                nc.gpsimd.memset(xv[:, 7 : XROWS - 7, 0:7], 0.0)   # lead cols
                nc.gpsimd.memset(xv[:, 7 : XROWS - 7, 7 + W :], 0.0)  # trail
                if s == 0:
                    nc.gpsimd.memset(xv[:, 7:10, :], 0.0)   # above-image pad
                else:
                    nc.gpsimd.memset(xv[:, 66:69, :], 0.0)  # below-image pad
                xbufs.append(xb)
                r = bpool.tile([CIN, RBF], BF16, tag=f"R{s}", name=f"R{s}")
                rbufs.append(r)
                qg = bpool.tile([CIN, UROWS * 118], BF16, tag=f"Qg{s}",
                                name=f"qg{s}")
                qgv = qg[:].rearrange("c (k g) -> c k g", g=118)
                nc.gpsimd.memset(qgv[:, :, 115:118], 0.0)  # x cols 112.. zeros
                qgbufs.append(qg)
                dv = bpool.tile([CIN, UROWS * W], BF16, tag=f"d3s{s}",
                                name=f"d3s{s}")
                dvbufs.append(dv)

            def issue_in_dma(unit):
                b, hh = unit // 2, unit % 2
                xv = xbufs[hh][:].rearrange("c (r q) -> c r q", q=PW2)
                xr0 = max(0, 56 * hh - 3)            # first valid x row
                dst0 = 10 if hh == 0 else 7          # its XB row
                n0 = 41 - dst0                       # split at XB row 41
                n1 = DROWS - n0
                slices = ((0, n0 // 2), (n0 // 2, n0 - n0 // 2),
                          (n0, n1 // 2), (n0 + n1 // 2, n1 - n1 // 2))
                for si, (s0, ln) in enumerate(slices):
                    # gpsimd (idle) instead of scalar: a 2.4us direct DMA on
                    # the scalar queue delays the activations that free PSUM
                    # banks for matmul groups 3-4
                    eng = nc.sync if si % 2 == 0 else nc.gpsimd
                    eng.dma_start(
                        xv[:, dst0 + s0 : dst0 + s0 + ln, 7 : 7 + W],
                        t_x[b, :, xr0 + s0 : xr0 + s0 + ln, :],
                    )

            _bank = [0]
            issue_in_dma(0)
            for unit in range(B_PER * 2):
                b, hh = unit // 2, unit % 2
                xb, Rs = xbufs[hh], rbufs[hh]
                qg = qgbufs[hh]
                dvs = dvbufs[hh]  # d3 tile
                xt = xb[:].rearrange("c (r q) -> c q r", q=PW2)  # [c,122,76]
                rv = Rs[:].rearrange("c (r q) -> c r q", q=PW2)  # rows 7..68
                qgv = qg[:].rearrange("c (k g) -> c k g", g=118)

                def r_chunk(ci):
                    r0, nrows = RCHUNKS[ci]
                    base = r0 * PW2
                    sbase = (r0 - 7) * PW2
                    ln = nrows * PW2 - 7
                    nc.vector._custom_dve(
                        opa,
                        out=Rs[:, sbase : sbase + ln],
                        in0=xb[:, base + 7 : base + 7 + ln],
                        in1=xb[:, base : base + ln],
                    )

                r_chunk(0)
                qpv = qp[:].rearrange("c (p k) -> c p k", k=QK)
                qpk = qp[:].rearrange("c (p k) -> c k p", k=QK)
                # Q scan in two page(column)-halves; relayout each half on
                # the scalar engine as soon as it lands; r0 first so the
                # R-taps give the tensor engine an early runway.
                nc.vector._custom_dve(
                    opa, out=qpv[:, 0:58, :],
                    in0=xt[:, 4:62, 7:XROWS], in1=xt[:, 4:62, 0:QK],
                )
                nc.scalar.copy(qgv[:, :, 0:58], qpk[:, 6:62, 0:58])
                nc.vector._custom_dve(
                    opa, out=qpv[:, 58:115, :],
                    in0=xt[:, 62:119, 7:XROWS], in1=xt[:, 62:119, 0:QK],
                )
                nc.scalar.copy(qgv[:, :, 58:115], qpk[:, 6:62, 58:115])
                r_chunk(1)
                # d3 via custom DVE op: the stock tensor_tensor writer gets
                # no dependency edge to the matmul readers (trace-proven
                # race); custom-DVE writes are tracked like the R/Q scans.
                # Two row-halves: the first half unblocks group 1's d3-tap
                # ~1.7us earlier without reordering anything across engines.
                d3v = dvs[:].rearrange("c (k j) -> c k j", j=W)
                nc.vector._custom_dve(
                    opd, out=d3v[:, 0:28, :],
                    in0=qgv[:, 0:28, 6:118], in1=qgv[:, 0:28, 0:112],
                )
                nc.vector._custom_dve(
                    opd, out=d3v[:, 28:56, :],
                    in0=qgv[:, 28:56, 6:118], in1=qgv[:, 28:56, 0:112],
                )
                # hoist next unit's input DMAs ahead of this unit's output
                # DMAs in the in-order Sync queue
                if unit + 1 < B_PER * 2:
                    issue_in_dma(unit + 1)

                # ---- 11-tap matmuls; 8/6-tile groups rotate PSUM banks;
                # the 7 R-taps of each group run while the Q->relayout->dv
                # chain of this unit finishes ----
                for t0, ntiles in ((0, 4), (4, 4), (8, 4), (12, 2)):
                    accs = []
                    for t in range(ntiles):
                        bk = _bank[0]
                        _bank[0] = (bk + 1) % 8
                        accs.append(
                            ppool.tile(
                                [COUT, OTF], F32, tag=f"acc{bk}", name=f"acc{bk}"
                            )
                        )
                    for tap in range(NTAPS):
                        wslice = wt[:, tap * COUT : (tap + 1) * COUT]
                        for t in range(ntiles):
                            i0 = (t0 + t) * OUT_ROWS  # unit-local out row
                            if tap < 7:
                                u = tap - 3
                                rr = 3 + i0 + u       # R-store row
                                rhs = rv[:, rr : rr + 4, 3 : 3 + W]
                            elif tap < 11:
                                v = (1, -1, 2, -2)[tap - 7]
                                rhs = qgv[:, i0 : i0 + 4, 3 + v : 115 + v]
                            else:
                                dvv = dvs[:].rearrange(
                                    "c (k j) -> c k j", j=W
                                )
                                rhs = dvv[:, i0 : i0 + 4, :]
                            nc.tensor.matmul(
                                accs[t][:],
                                wslice,
                                rhs,
                                start=(tap == 0),
                                stop=(tap == NTAPS - 1),
                            )
                    for t in range(ntiles):
                        i0 = (t0 + t) * OUT_ROWS
                        ot = opool.tile([COUT, OTF], BF16, tag="ot", name="ot")
                        nc.scalar.activation(
                            ot[:], accs[t][:], AF.Identity,
                            bias=bias_sb[:], scale=1.0,
                        )
                        nc.sync.dma_start(
                            t_out[
                                b, :, 56 * hh + i0 : 56 * hh + i0 + OUT_ROWS, :
                            ].rearrange("o r j -> o (r j)"),
                            ot[:],
                        )

    nc.compile()
    return nc


def _make_in_maps(x, abc, bias):
    A, Bm, Cc = abc[0:128], abc[128:256], abc[256:384]
    taps = [u * A + Cc for u in range(-3, 4)] + [
        Bm, -Bm, 2 * Bm, -2 * Bm, 3 * Bm]
    wts = np.ascontiguousarray(np.stack(taps), dtype=np.float32)
    bias2 = np.ascontiguousarray(bias.reshape(COUT, 1), dtype=np.float32)
    return [
        {
            "xs": np.ascontiguousarray(x[c * B_PER : (c + 1) * B_PER]),
            "wts": wts,
            "bias": bias2,
        }
        for c in range(N_CORES)
    ]


def kernel(x: np.ndarray, abc: np.ndarray, bias: np.ndarray) -> np.ndarray:
    x = np.ascontiguousarray(x, dtype=np.float32)
    abc = np.asarray(abc, dtype=np.float32)
    bias = np.asarray(bias, dtype=np.float32)

    if "nc" not in _CACHE:
        _CACHE["nc"] = _build()
    nc = _CACHE["nc"]

    in_maps = _make_in_maps(x, abc, bias)
    res = run_bass_kernel_spmd(nc, in_maps, list(range(N_CORES)))
    out = np.concatenate(
        [np.asarray(res.results[c]["out"]) for c in range(N_CORES)], axis=0
    )
    return out.astype(np.float32)


if __name__ == "__main__":
    rng = np.random.default_rng(0)
    x = rng.standard_normal((16, 128, 112, 112), dtype=np.float32)
    abc = (rng.standard_normal((384, 128)) * 0.05).astype(np.float32)
    bias = (rng.standard_normal((128,)) * 0.05).astype(np.float32)
    out = kernel(x=x, abc=abc, bias=bias)
    print(out.shape, out.dtype)
